# revision 89
# baseline (speedup 1.0000x reference)
"""Trainium2 Bass kernel for EquivariantSelfAttentionBlock.

Sharding (8 NeuronCores, pure SPMD, no collectives):
  core c -> (batch b = c//4, query-slice t = c%4 of 256 queries).
  Each core gets the full `a` of its batch, row-permuted so its own 256
  queries come first.  It computes LayerNorm + k/v for all 1024 keys
  (replicated inside the 4-core batch group) and everything downstream
  only for its 256 queries.

Math restructuring (host-side prep, O(N*small) only):
  * LayerNorm affine + attention scale folded into Wq/Wk/Wv/biases.
  * RFF pair embedding expanded with the trig identity so the pair-MLP
    first layer becomes a K=32 matmul: z1[q,k,:] = RQ[:,(q,:)]^T Bk[:,k].
  * gaussian window + |p|^2 logit terms as one K=4 matmul (distK/distQ8),
    in bf16 (fp32 matmuls run at 1/4 PE rate).
  * biasK dropped entirely (q.bk is constant along the softmax axis);
    bv folded into the residual (softmax weights sum to 1).
  * pair-MLP second layer as col-tiled block-diag matmul over groups of
    4 queries; output DMA-transposed to k-major and injected into the
    logit PSUM via an identity matmul.
  * softmax without max subtraction (logits <= ~1 by construction).
"""

import sys

if "/opt/trn_rl_repo" not in sys.path:
    sys.path.insert(0, "/opt/trn_rl_repo")

import numpy as np
import ml_dtypes

import concourse.bass as bass
import concourse.mybir as mybir
import concourse.tile as tile
from concourse import bacc
from concourse.masks import make_identity

F32 = mybir.dt.float32
BF16 = mybir.dt.bfloat16
FP8 = mybir.dt.float8e4
AF = mybir.ActivationFunctionType
ALU = mybir.AluOpType

B, N, C = 2, 1024, 256
H, DH = 8, 256
F, PH = 16, 32
EPS = 1e-6
HDH = H * DH            # 2048
NQ = 256                # queries per core
NCORES = 8
P = 128
NG = NQ // 4            # 64 groups of 4 queries
NPAIR = NG // 2         # 32 row-packed group pairs
NTAU = NG // 4          # 16 quads (4 groups) for z2 col-packing
NKT = N // P            # 8 key tiles
NDC = HDH // P          # 16 head-dim chunks

_BF = ml_dtypes.bfloat16

# jax.nn.gelu defaults to the tanh approximation; CoreSim only implements
# the exact variant, so tests override this with AF.Gelu.
GELU = AF.Gelu_apprx_tanh

# phase-name -> first instruction id, filled during _body for profiling
PHASE_MARKS = []


def _mark(nc, name):
    PHASE_MARKS.append((name, nc.next_id()))


# ---------------------------------------------------------------------------
# device program (identical on all 8 cores)
# ---------------------------------------------------------------------------

def _build_program(n_iters: int = 1):
    nc = bacc.Bacc("TRN2", target_bir_lowering=False, debug=False,
                   num_devices=NCORES)

    def din(name, shape, dt=F32):
        return nc.dram_tensor(name, list(shape), dt, kind="ExternalInput").ap()

    d = {
        "a_full": din("a_full", [N, C]),
        "Wq": din("Wq", [C, HDH], BF16),
        "Wk": din("Wk", [C, HDH], BF16),
        "Wv": din("Wv", [C, HDH], BF16),
        "biasQ": din("biasQ", [P, NDC]),
        "RQ2": din("RQ2", [64, NPAIR * P], BF16),
        "BkT2": din("BkT2", [64, N], BF16),
        "W2bd": din("W2bd", [P, 32], BF16),
        "bp1p": din("bp1p", [P, 1]),
        "bp2p": din("bp2p", [P, 1]),
        "distK": din("distK", [4, N], BF16),
        "distQ8": din("distQ8", [4, H * NQ], BF16),
        "Wo": din("Wo", [HDH, C], BF16),
        "apb": din("apb", [NQ, C]),
        "Wf1": din("Wf1", [C, C], BF16),
        "Wf2": din("Wf2", [C, C], BF16),
        "bf1p": din("bf1p", [P, C // P]),
        "bf2r": din("bf2r", [1, C]),
    }
    d["rden_dram"] = nc.dram_tensor("rden_dram", [1, H * NQ], BF16,
                                    kind="Internal").ap()
    out_d = nc.dram_tensor("out", [NQ, C], F32, kind="ExternalOutput").ap()

    with tile.TileContext(nc) as tc:
        for _ in range(n_iters):
            _body(nc, tc, d, out_d)
    nc.compile()
    return nc


def _body(nc, tc, d, out_d):
    dma = nc.sync.dma_start

    from contextlib import ExitStack
    octx = ExitStack()
    pp = octx.enter_context(tc.tile_pool(name="persist", bufs=1))

    def T(shape, dtype, name):
        return pp.tile(shape, dtype, tag=name, name=name)

    # ---- persistent SBUF arrays -------------------------------------------
    kT = T([P, NDC * N], BF16, "kT_sb")          # [dh-chunk i][:, i*N + k]
    qT = T([P, NDC * NQ], BF16, "qT_sb")         # [:, i*NQ + q]
    vN = T([P, NKT * HDH], BF16, "v_sb")         # [kt][:, kt*HDH + hd]
    attT = T([P, NKT * 2048], BF16, "attT_sb")   # per kt: h*256 + q

    ident_f = T([P, P], F32, "ident_f")
    ident_b = T([P, P], BF16, "ident_b")
    make_identity(nc, ident_f)
    make_identity(nc, ident_b)

    ones16 = T([P, 1], BF16, "ones16")           # den lhsT: cancels sigma_v=16
    nc.vector.memset(ones16, 16.0)
    epsT = T([P, 1], F32, "epsT")
    nc.vector.memset(epsT, EPS)


    # z2T lives from phase 2 until the end of the logits loop
    z2ctx = ExitStack()
    z2pp = z2ctx.enter_context(tc.tile_pool(name="z2t_pool", bufs=1))
    z2T = z2pp.tile([P, NKT * 2048], BF16, tag="z2T_sb", name="z2T_sb")

    # ---- phases 0-2 in ONE scope: LayerNorm + q/k/v projections + pair
    # MLP share PSUM pools that coexist, so Tile can overlap the PE-bound
    # projections with the ACT-bound pair-MLP gelu stream.
    with tc.tile_pool(name="p01_sbuf", bufs=2) as sb, \
         tc.tile_pool(name="p01_anT", bufs=1) as anp, \
         tc.tile_pool(name="p01_w", bufs=1) as wpool, \
         tc.tile_pool(name="p2_sbuf", bufs=1) as sb2, \
         tc.tile_pool(name="p2_g", bufs=4) as gpool, \
         tc.tile_pool(name="p2_z2e", bufs=2) as z2epool, \
         tc.tile_pool(name="p01_psum", bufs=2, space="PSUM") as ps, \
         tc.tile_pool(name="p2_z1p", bufs=2, space="PSUM") as z1ps, \
         tc.tile_pool(name="p2_z2p", bufs=1, space="PSUM") as z2ps:

        # pair-MLP inputs first on the gpsimd queue (z1 is early PE work)
        rq_sb = sb2.tile([64, NPAIR * P], BF16, name="rq_sb")
        nc.gpsimd.dma_start(out=rq_sb, in_=d["RQ2"])
        bkt_sb = sb2.tile([64, N], BF16, name="bkt_sb")
        nc.gpsimd.dma_start(out=bkt_sb, in_=d["BkT2"])
        w2bd_sb = sb2.tile([P, 32], BF16, name="w2bd_sb")
        dma(out=w2bd_sb, in_=d["W2bd"])
        bp1p_sb = sb2.tile([P, 1], F32, name="bp1p_sb")
        dma(out=bp1p_sb, in_=d["bp1p"])
        bp2p_sb = sb2.tile([P, 1], F32, name="bp2p_sb")
        dma(out=bp2p_sb, in_=d["bp2p"])

        anT = anp.tile([P, 2 * N], BF16, tag="anT", name="anT_sb")
        wq_sb = wpool.tile([P, 2 * HDH], BF16, name="wq_sb")
        wk_sb = wpool.tile([P, 2 * HDH], BF16, name="wk_sb")
        wv_sb = wpool.tile([P, 2 * HDH], BF16, name="wv_sb")
        for ct in range(2):
            nc.gpsimd.dma_start(out=wk_sb[:, ct * HDH:(ct + 1) * HDH],
                                in_=d["Wk"][ct * P:(ct + 1) * P, :])
            nc.gpsimd.dma_start(out=wq_sb[:, ct * HDH:(ct + 1) * HDH],
                                in_=d["Wq"][ct * P:(ct + 1) * P, :])
            nc.gpsimd.dma_start(out=wv_sb[:, ct * HDH:(ct + 1) * HDH],
                                in_=d["Wv"][ct * P:(ct + 1) * P, :])
        bq_sb = wpool.tile([P, NDC], F32, name="bq_sb")
        dma(out=bq_sb, in_=d["biasQ"])

        # LayerNorm (natural layout) + transpose into anT (bf16)
        _mark(nc, 'LN')
        with tc.tile_pool(name="a_pool", bufs=1) as apool:
            af = d["a_full"]
            for hh in range(2):
                a_half = apool.tile([P, NKT // 2 * C], F32, tag="a_all",
                                    name="a_half")
                nc.sync.dma_start(
                    out=a_half,
                    in_=bass.AP(tensor=af.tensor,
                                offset=af.offset + hh * (N // 2) * C,
                                ap=[[C, P], [P * C, NKT // 2], [1, C]]))
                for nt4 in range(NKT // 2):
                    nt = hh * 4 + nt4
                    at = a_half[:, nt4 * C:(nt4 + 1) * C]
                    stats = sb.tile([P, 6], F32, tag="stats")
                    nc.vector.bn_stats(out=stats, in_=at)
                    mv = sb.tile([P, 2], F32, tag="mv")
                    nc.vector.bn_aggr(out=mv, in_=stats)
                    std = sb.tile([P, 1], F32, tag="std")
                    nc.scalar.activation(out=std, in_=mv[:, 1:2], func=AF.Sqrt,
                                         bias=epsT, scale=1.0)
                    rstd = sb.tile([P, 1], F32, tag="rstd")
                    nc.vector.reciprocal(out=rstd, in_=std)
                    an = sb.tile([P, C], BF16, tag="an_t")
                    nc.vector.tensor_scalar(out=an, in0=at, scalar1=mv[:, 0:1],
                                            scalar2=rstd, op0=ALU.subtract,
                                            op1=ALU.mult)
                    # xbar transpose: chunk ct lands at anT[:, ct*N + nt*128]
                    asl = anT[:, nt * P: nt * P + 1]
                    aob = bass.AP(tensor=asl.tensor, offset=asl.offset,
                                  ap=[asl.ap[0], [N, 2], [1, P]])
                    nc.sync.dma_start_transpose(aob, an)

        # ---- pair MLP -> z2T: emitted BEFORE the projections so the PE
        # starts on z1 immediately (projections wait on LayerNorm anyway).
        _mark(nc, 'pair')
        g_tiles = [None] * 4  # rotating per quad
        for tau in range(NTAU):
            for pj in range(2):           # two row-packed pairs per quad
                pi = tau * 2 + pj         # pair index
                for j in range(2):
                    g = gpool.tile([P, N], BF16, tag="g")
                    z1p = z1ps.tile([P, N], F32, tag="z1", name="z1p")
                    for nk in range(2):
                        nc.tensor.matmul(
                            z1p[:, nk * 512:(nk + 1) * 512],
                            lhsT=rq_sb[32 * j:32 * (j + 1), pi * P:(pi + 1) * P],
                            rhs=bkt_sb[32 * j:32 * (j + 1),
                                       nk * 512:(nk + 1) * 512],
                            start=True, stop=True,
                            tile_position=(32 * j, 0))
                    nc.scalar.activation(out=g, in_=z1p,
                                         func=GELU,
                                         bias=bp1p_sb, scale=1.0 / 32.0)
                    g_tiles[pj * 2 + j] = g
            z2p = z2ps.tile([P, N], F32, tag="z2p")
            for j in range(4):
                for nk in range(2):
                    nc.tensor.matmul(
                        z2p[32 * j:32 * (j + 1), nk * 512:(nk + 1) * 512],
                        lhsT=w2bd_sb,
                        rhs=g_tiles[j][:, nk * 512:(nk + 1) * 512],
                        start=True, stop=True,
                        tile_position=(0, 32 * j),
                        skip_group_check=True)
            z2e = z2epool.tile([P, N], BF16, tag="z2e")
            nc.vector.tensor_scalar_add(out=z2e, in0=z2p, scalar1=bp2p_sb)
            # one xbar transpose scatters all 8 k-chunks: out chunk kt is
            # z2T[:, kt*2048 + tau*128 : ... + 128] = z2e[:, kt*128:...].T
            zsl = z2T[:, tau * P: tau * P + 1]
            zob = bass.AP(tensor=zsl.tensor, offset=zsl.offset,
                          ap=[zsl.ap[0], [2048, NKT], [1, P]])
            nc.sync.dma_start_transpose(zob, z2e)

        # kT: per dh-chunk i -> [128, N]   (no bias: q.bk is constant over the
        # softmax axis, so biasK drops out of the attention exactly)
        _mark(nc, 'kT')
        for i in range(NDC):
            for nk in range(2):
                kp = ps.tile([P, 512], F32, tag="proj", name="kp")
                for ct in range(2):
                    nc.tensor.matmul(
                        kp,
                        lhsT=wk_sb[:, ct * HDH + i * P: ct * HDH + (i + 1) * P],
                        rhs=anT[:, ct * N + nk * 512: ct * N + (nk + 1) * 512],
                        start=(ct == 0), stop=(ct == 1))
                nc.vector.tensor_copy(
                    out=kT[:, i * N + nk * 512: i * N + (nk + 1) * 512],
                    in_=kp)

        # qT: per dh-chunk i -> [128, NQ]  (queries are first NQ columns)
        _mark(nc, 'qT')
        for i in range(NDC):
            qp = ps.tile([P, NQ], F32, tag="proj", name="qp")
            for ct in range(2):
                nc.tensor.matmul(
                    qp, lhsT=wq_sb[:, ct * HDH + i * P: ct * HDH + (i + 1) * P],
                    rhs=anT[:, ct * N: ct * N + NQ],
                    start=(ct == 0), stop=(ct == 1))
            nc.vector.tensor_scalar_add(
                out=qT[:, i * NQ:(i + 1) * NQ], in0=qp,
                scalar1=bq_sb[:, i:i + 1])

        # v (natural layout): per key tile kt -> [128, HDH]  (bv is folded into
        # apb host-side: softmax weights sum to 1, so att@(v+bv) = att@v + bv)
        _mark(nc, 'v')
        for kt in range(NKT):
            for dq in range(4):
                vp = ps.tile([P, 512], F32, tag="proj", name="vp")
                for ct in range(2):
                    nc.tensor.matmul(
                        vp,
                        lhsT=anT[:, ct * N + kt * P: ct * N + (kt + 1) * P],
                        rhs=wv_sb[:, ct * HDH + dq * 512:
                                  ct * HDH + (dq + 1) * 512],
                        start=(ct == 0), stop=(ct == 1))
                nc.vector.tensor_copy(
                    out=vN[:, kt * HDH + dq * 512: kt * HDH + (dq + 1) * 512],
                    in_=vp)

    # ---- phase 3: logits, softmax, AV ------------------------------------
    with tc.tile_pool(name="p3_sbuf", bufs=1) as sb3:
        distK_sb = sb3.tile([4, N], BF16, name="distK_sb")
        dma(out=distK_sb, in_=d["distK"])
        distQ8_sb = sb3.tile([4, H * NQ], BF16, name="distQ8_sb")
        dma(out=distQ8_sb, in_=d["distQ8"])

        _mark(nc, 'logits')
        with tc.tile_pool(name="p3_qp", bufs=2, space="PSUM") as qps:
            for kt in range(NKT):
                Qp = qps.tile([P, H * NQ], F32, tag="Qp")
                for cch in range(4):
                    nc.tensor.matmul(
                        Qp[:, cch * 512:(cch + 1) * 512],
                        lhsT=distK_sb[:, kt * P:(kt + 1) * P],
                        rhs=distQ8_sb[:, cch * 512:(cch + 1) * 512],
                        start=True, stop=False)
                for h in range(H):
                    zr = z2T[:, kt * 2048 + 4 * h: kt * 2048 + 4 * h + 1]
                    zap = bass.AP(
                        tensor=zr.tensor, offset=zr.offset,
                        ap=[zr.ap[0], [128, NTAU], [32, 4], [1, 4]])
                    nc.tensor.matmul(
                        Qp[:, h * NQ:(h + 1) * NQ], lhsT=ident_b, rhs=zap,
                        start=False, stop=False)
                for h in range(H):
                    for i2 in range(2):
                        i = h * 2 + i2
                        nc.tensor.matmul(
                            Qp[:, h * NQ:(h + 1) * NQ],
                            lhsT=kT[:, i * N + kt * P: i * N + (kt + 1) * P],
                            rhs=qT[:, i * NQ:(i + 1) * NQ],
                            start=False, stop=(i2 == 1 and h % 2 == 1))
                nc.scalar.activation(
                    out=attT[:, kt * 2048:(kt + 1) * 2048], in_=Qp,
                    func=AF.Exp, bias=0.0, scale=1.0 / 1024.0)

    z2ctx.close()
    avp = octx.enter_context(tc.tile_pool(name="av_persist", bufs=1))
    oT = avp.tile([P, NDC * NQ], BF16, tag="oT_sb", name="oT_sb")
    rden = avp.tile([1, H * NQ], BF16, tag="rden", name="rden_sb")
    rdb = avp.tile([P, H * NQ], BF16, tag="rdb", name="rdb_sb")
    # tail weights, loaded here so the DMA hides under den/AV
    wo_sb = avp.tile([P, NDC * C], BF16, tag="wo_sb", name="wo_sb")
    wod = d["Wo"]
    nc.gpsimd.dma_start(
        out=wo_sb, in_=bass.AP(tensor=wod.tensor, offset=wod.offset,
                               ap=[[C, P], [P * C, NDC], [1, C]]))
    wf1_sb = avp.tile([P, 2 * C], BF16, tag="wf1_sb", name="wf1_sb")
    wf2_sb = avp.tile([P, 2 * C], BF16, tag="wf2_sb", name="wf2_sb")
    for wname, wsb in (("Wf1", wf1_sb), ("Wf2", wf2_sb)):
        wd = d[wname]
        nc.gpsimd.dma_start(
            out=wsb, in_=bass.AP(tensor=wd.tensor, offset=wd.offset,
                                 ap=[[C, P], [P * C, 2], [1, C]]))

    _mark(nc, 'den_av')
    with tc.tile_pool(name="p3_den", bufs=1, space="PSUM") as denps, \
         tc.tile_pool(name="p3_av", bufs=4, space="PSUM") as avps:
        denp = denps.tile([1, H * NQ], F32, name="denp")
        for cc in range(4):
            for kt in range(NKT):
                nc.tensor.matmul(
                    denp[:, cc * 512:(cc + 1) * 512], lhsT=ones16,
                    rhs=attT[:, kt * 2048 + cc * 512: kt * 2048 + (cc + 1) * 512],
                    start=(kt == 0), stop=(kt == NKT - 1))
        with nc.allow_low_precision(reason="softmax denom bcast in bf16"):
            nc.vector.reciprocal(out=rden, in_=denp)
        rdd = d["rden_dram"]
        dma(out=rdd, in_=rden)
        nc.gpsimd.dma_start(
            out=rdb, in_=bass.AP(tensor=rdd.tensor, offset=rdd.offset,
                                 ap=[[0, P], [1, H * NQ]]))

        for h in range(H):
            for dhh in range(2):
                i = h * 2 + dhh
                op = avps.tile([P, NQ], F32, tag="op")
                for kt in range(NKT):
                    nc.tensor.matmul(
                        op,
                        lhsT=vN[:, kt * HDH + h * DH + dhh * P:
                                kt * HDH + h * DH + (dhh + 1) * P],
                        rhs=attT[:, kt * 2048 + h * NQ:
                                 kt * 2048 + (h + 1) * NQ],
                        start=(kt == 0), stop=(kt == NKT - 1))
                nc.vector.tensor_tensor(
                    out=oT[:, i * NQ:(i + 1) * NQ], in0=op,
                    in1=rdb[:, h * NQ:(h + 1) * NQ], op=ALU.mult)

    _mark(nc, 'tail')
    # ---- phase 4: output projection + residual + FFN ---------------------
    with tc.tile_pool(name="p4_sbuf", bufs=1) as sb4, \
         tc.tile_pool(name="p4_ps", bufs=2, space="PSUM") as ps4, \
         tc.tile_pool(name="p4_pst", bufs=4, space="PSUM") as pst4:

        apb_sb = sb4.tile([P, 2 * C], F32, name="apb_sb")
        apd = d["apb"]
        nc.gpsimd.dma_start(
            out=apb_sb, in_=bass.AP(tensor=apd.tensor, offset=apd.offset,
                                    ap=[[C, P], [P * C, 2], [1, C]]))
        bf1p_sb = sb4.tile([P, C // P], F32, name="bf1p_sb")
        dma(out=bf1p_sb, in_=d["bf1p"])
        bf2b = sb4.tile([P, C], F32, name="bf2b")
        srcb = d["bf2r"]
        nc.gpsimd.dma_start(
            out=bf2b, in_=bass.AP(tensor=srcb.tensor, offset=srcb.offset,
                                  ap=[[0, P], [1, C]]))

        res = sb4.tile([P, 2 * C], F32, name="res_sb")      # [qh][:, qh*C + c]
        for qh in range(2):
            prj = ps4.tile([P, C], F32, tag="p4")
            for i in range(NDC):
                nc.tensor.matmul(
                    prj, lhsT=oT[:, i * NQ + qh * P: i * NQ + (qh + 1) * P],
                    rhs=wo_sb[:, i * C:(i + 1) * C],
                    start=(i == 0), stop=(i == NDC - 1))
            nc.vector.tensor_tensor(
                out=res[:, qh * C:(qh + 1) * C], in0=prj,
                in1=apb_sb[:, qh * C:(qh + 1) * C], op=ALU.add)

        resT = sb4.tile([P, 2 * NQ], BF16, name="resT_sb")  # [ct][:, ct*NQ + q]
        for qh in range(2):
            for ct in range(2):
                tp4 = pst4.tile([P, P], F32, tag="tp4")
                nc.tensor.transpose(
                    tp4, res[:, qh * C + ct * P: qh * C + (ct + 1) * P],
                    ident_f)
                nc.vector.tensor_copy(
                    out=resT[:, ct * NQ + qh * P: ct * NQ + (qh + 1) * P],
                    in_=tp4)

        gT = sb4.tile([P, 2 * NQ], BF16, name="gT_sb")      # [cc][:, cc*NQ + q]
        for cc in range(2):
            fp = ps4.tile([P, NQ], F32, tag="p4")
            for ct in range(2):
                nc.tensor.matmul(
                    fp, lhsT=wf1_sb[:, ct * C + cc * P: ct * C + (cc + 1) * P],
                    rhs=resT[:, ct * NQ:(ct + 1) * NQ],
                    start=(ct == 0), stop=(ct == 1))
            nc.scalar.activation(out=gT[:, cc * NQ:(cc + 1) * NQ], in_=fp,
                                 func=GELU,
                                 bias=bf1p_sb[:, cc:cc + 1], scale=1.0)

        for qh in range(2):
            f2 = ps4.tile([P, C], F32, tag="p4")
            for cc in range(2):
                nc.tensor.matmul(
                    f2, lhsT=gT[:, cc * NQ + qh * P: cc * NQ + (qh + 1) * P],
                    rhs=wf2_sb[:, cc * C:(cc + 1) * C],
                    start=(cc == 0), stop=(cc == 1))
            ot = sb4.tile([P, C], F32, tag="ot")
            nc.vector.tensor_tensor(out=ot, in0=f2, in1=bf2b, op=ALU.add)
            dma(out=out_d[qh * P:(qh + 1) * P, :], in_=ot)

    octx.close()


# ---------------------------------------------------------------------------
# host-side input prep
# ---------------------------------------------------------------------------

def _prep_core_inputs(inputs):
    f32 = np.float32
    p = np.asarray(inputs["p"], f32)
    a = np.asarray(inputs["a"], f32)
    sigma = float(np.asarray(inputs["window_size"]).reshape(-1)[0])
    ln_s = np.asarray(inputs["ln_scale"], f32)
    ln_b = np.asarray(inputs["ln_bias"], f32)
    Wq, bq = np.asarray(inputs["Wq"], f32), np.asarray(inputs["bq"], f32)
    Wk, bk = np.asarray(inputs["Wk"], f32), np.asarray(inputs["bk"], f32)
    Wv, bv = np.asarray(inputs["Wv"], f32), np.asarray(inputs["bv"], f32)
    rff_B = np.asarray(inputs["rff_B"], f32)
    Wp1, bp1 = np.asarray(inputs["Wp1"], f32), np.asarray(inputs["bp1"], f32)
    Wp2, bp2 = np.asarray(inputs["Wp2"], f32), np.asarray(inputs["bp2"], f32)
    Wo, bo = np.asarray(inputs["Wo"], f32), np.asarray(inputs["bo"], f32)
    Wf1, bf1 = np.asarray(inputs["Wf1"], f32), np.asarray(inputs["bf1"], f32)
    Wf2, bf2 = np.asarray(inputs["Wf2"], f32), np.asarray(inputs["bf2"], f32)

    # operand scaling: q,k carry sigma_q=sigma_k=8, v carries sigma_v=16.
    # The logits PSUM is then LAM=1024x the true logits (1/sqrt(DH)=1/16
    # folded in), compensated by the Exp activation's scale=1/LAM.  The
    # denominator matmul uses lhsT=16 so op*rdb cancels sigma_v exactly.
    SQ, SK, SV = 8.0, 8.0, 16.0
    LAM = SQ * SK * np.sqrt(f32(DH))       # 1024
    Wq_f = (ln_s[:, None] * Wq) * SQ
    bq_f = (bq + ln_b @ Wq) * SQ
    Wk_f = (ln_s[:, None] * Wk) * SK
    Wv_f = (ln_s[:, None] * Wv) * SV
    bv_f = bv + ln_b @ Wv

    u = 2.0 * np.pi * (p @ rff_B)          # [B, N, F]
    su, cu = np.sin(u), np.cos(u)
    pn2 = (p ** 2).sum(-1)                 # [B, N]

    # W2 block-diag: rows (ql*32+h1), cols (h*4+ql), scaled by LAM
    W2bd = np.zeros((P, 32), f32)
    for ql in range(4):
        for h1 in range(PH):
            for h in range(H):
                W2bd[ql * 32 + h1, h * 4 + ql] = Wp2[h1, h] * LAM
    bp1p = np.tile(bp1, 4).reshape(P, 1).astype(f32)
    bp2p = (np.tile(np.repeat(bp2, 4), 4) * LAM).reshape(P, 1).astype(f32)
    bf1p = bf1.reshape(C // P, P).T.copy()
    biasQ = bq_f.reshape(NDC, P).T.copy()

    shared = {
        "Wq": Wq_f.astype(_BF), "Wk": Wk_f.astype(_BF),
        "Wv": Wv_f.astype(_BF),
        "biasQ": biasQ,
        "W2bd": W2bd.astype(_BF), "bp1p": bp1p, "bp2p": bp2p,
        "Wo": Wo.astype(_BF),
        "Wf1": Wf1.astype(_BF), "Wf2": Wf2.astype(_BF),
        "bf1p": bf1p, "bf2r": bf2.reshape(1, C),
    }

    in_maps = []
    for c in range(NCORES):
        b, t = c // 4, c % 4
        perm = (np.arange(N) + t * NQ) % N
        a_p = a[b][perm]
        su_p, cu_p = su[b][perm], cu[b][perm]
        p_p, pn2_p = p[b][perm], pn2[b][perm]

        # k-side of z1 trig expansion: rows j'=f -> cos, j'=F+f -> sin
        BkT = np.concatenate([cu_p.T, su_p.T], 0)       # [32, N]
        BkT2 = np.concatenate([BkT, BkT], 0)            # [64, N] row-packed x2

        # q-side: R[q, j', h1], pre-scaled by MU=32 (the gelu reads the z1
        # PSUM with scale=1/MU; harmless in bf16, needed if z1 goes fp8)
        MU = 32.0
        suq, cuq = su_p[:NQ], cu_p[:NQ]                 # [NQ, F]
        Ra = (np.einsum("qf,fh->qfh", suq, Wp1[:F]) +
              np.einsum("qf,fh->qfh", cuq, Wp1[F:]))
        Rb = (np.einsum("qf,fh->qfh", -cuq, Wp1[:F]) +
              np.einsum("qf,fh->qfh", suq, Wp1[F:]))
        R = np.concatenate([Ra, Rb], 1) * MU            # [NQ, 32, PH]
        # RQ2[32j+j', pi*128 + ql*32 + h1], q = pi*8 + j*4 + ql
        RQ2 = (R.reshape(NPAIR, 2, 4, 32, PH)
               .transpose(1, 3, 0, 2, 4).reshape(64, NPAIR * P))

        # rank-4 gaussian-window term in bf16 (fp32 matmuls are 4x slower);
        # row 3 keeps exp() <= e^~1 per row
        inv2s = 1.0 / (2.0 * sigma * sigma)
        distK = np.stack([p_p[:, 0], p_p[:, 1], pn2_p,
                          np.ones(N, f32)], 0)          # [4, N]
        distQ = np.stack([p_p[:NQ, 0] * (2.0 * inv2s * LAM),
                          p_p[:NQ, 1] * (2.0 * inv2s * LAM),
                          -np.full(NQ, inv2s * LAM, f32),
                          -pn2_p[:NQ] * inv2s * LAM], 0)  # [4, NQ]
        distQ8 = np.tile(distQ, (1, H))                 # col h*NQ+q

        m = dict(shared)
        m.update({
            "a_full": np.ascontiguousarray(a_p),
            "RQ2": RQ2.astype(_BF), "BkT2": np.ascontiguousarray(BkT2).astype(_BF),
            "distK": np.ascontiguousarray(distK).astype(_BF),
            "distQ8": np.ascontiguousarray(distQ8).astype(_BF),
            "apb": np.ascontiguousarray(a_p[:NQ] + bo + bv_f @ Wo),
        })
        in_maps.append({k: np.ascontiguousarray(v) for k, v in m.items()})
    return in_maps


# ---------------------------------------------------------------------------
# entry point
# ---------------------------------------------------------------------------

_NC_CACHE = None


def _get_nc():
    global _NC_CACHE
    if _NC_CACHE is None:
        _NC_CACHE = _build_program()
    return _NC_CACHE


def kernel(**inputs):
    from concourse import bass_utils
    in_maps = _prep_core_inputs(inputs)
    nc = _get_nc()
    res = bass_utils.run_bass_kernel_spmd(nc, in_maps,
                                          core_ids=list(range(NCORES)))
    out = np.empty((B, N, C), np.float32)
    for c in range(NCORES):
        b, t = c // 4, c % 4
        out[b, t * NQ:(t + 1) * NQ, :] = res.results[c]["out"]
    return out


# revision 94
# speedup vs baseline: 1.1112x; 1.1112x over previous
"""Trainium2 Bass kernel for EquivariantSelfAttentionBlock.

Sharding (8 NeuronCores, pure SPMD, no collectives):
  core c -> (batch b = c//4, query-slice t = c%4 of 256 queries).
  Each core gets the full `a` of its batch, row-permuted so its own 256
  queries come first.  It computes LayerNorm + k/v for all 1024 keys
  (replicated inside the 4-core batch group) and everything downstream
  only for its 256 queries.

Math restructuring (host-side prep, O(N*small) only):
  * LayerNorm affine + attention scale folded into Wq/Wk/Wv/biases.
  * RFF pair embedding expanded with the trig identity so the pair-MLP
    first layer becomes a K=32 matmul: z1[q,k,:] = RQ[:,(q,:)]^T Bk[:,k].
  * gaussian window + |p|^2 logit terms as one K=4 matmul (distK/distQ8),
    in bf16 (fp32 matmuls run at 1/4 PE rate).
  * biasK dropped entirely (q.bk is constant along the softmax axis);
    bv folded into the residual (softmax weights sum to 1).
  * pair-MLP second layer as col-tiled block-diag matmul over groups of
    4 queries; output DMA-transposed to k-major and injected into the
    logit PSUM via an identity matmul.
  * softmax without max subtraction (logits <= ~1 by construction).
"""

import sys

if "/opt/trn_rl_repo" not in sys.path:
    sys.path.insert(0, "/opt/trn_rl_repo")

import numpy as np
import ml_dtypes

import concourse.bass as bass
import concourse.mybir as mybir
import concourse.tile as tile
from concourse import bacc
from concourse.masks import make_identity

F32 = mybir.dt.float32
BF16 = mybir.dt.bfloat16
FP8 = mybir.dt.float8e4
AF = mybir.ActivationFunctionType
ALU = mybir.AluOpType

B, N, C = 2, 1024, 256
H, DH = 8, 256
F, PH = 16, 32
EPS = 1e-6
HDH = H * DH            # 2048
NQ = 256                # queries per core
NCORES = 8
P = 128
NG = NQ // 4            # 64 groups of 4 queries
NPAIR = NG // 2         # 32 row-packed group pairs
NTAU = NG // 4          # 16 quads (4 groups) for z2 col-packing
NKT = N // P            # 8 key tiles
NDC = HDH // P          # 16 head-dim chunks

_BF = ml_dtypes.bfloat16

# jax.nn.gelu defaults to the tanh approximation; CoreSim only implements
# the exact variant, so tests override this with AF.Gelu.
GELU = AF.Gelu_apprx_tanh

# phase-name -> first instruction id, filled during _body for profiling
PHASE_MARKS = []


def _mark(nc, name):
    PHASE_MARKS.append((name, nc.next_id()))


# ---------------------------------------------------------------------------
# device program (identical on all 8 cores)
# ---------------------------------------------------------------------------

def _build_program(n_iters: int = 1):
    nc = bacc.Bacc("TRN2", target_bir_lowering=False, debug=False,
                   num_devices=NCORES)

    def din(name, shape, dt=F32):
        return nc.dram_tensor(name, list(shape), dt, kind="ExternalInput").ap()

    d = {
        "a_full": din("a_full", [N, C]),
        "Wq": din("Wq", [C, HDH], BF16),
        "Wk": din("Wk", [C, HDH], BF16),
        "Wv": din("Wv", [C, HDH], BF16),
        "biasQ": din("biasQ", [P, NDC]),
        "RQ2": din("RQ2", [64, NPAIR * P], BF16),
        "BkT2": din("BkT2", [64, N], BF16),
        "W2bd": din("W2bd", [P, 32], BF16),
        "bp1p": din("bp1p", [P, 1]),
        "bp2p": din("bp2p", [P, 1]),
        "distK": din("distK", [4, N], BF16),
        "distQ8": din("distQ8", [4, H * NQ], BF16),
        "Wo": din("Wo", [HDH, C], BF16),
        "apb": din("apb", [NQ, C]),
        "Wf1": din("Wf1", [C, C]),
        "Wf2": din("Wf2", [C, C]),
        "bf1p": din("bf1p", [P, C // P]),
        "bf2r": din("bf2r", [1, C]),
    }
    d["rden_dram"] = nc.dram_tensor("rden_dram", [1, H * NQ], BF16,
                                    kind="Internal").ap()
    out_d = nc.dram_tensor("out", [NQ, C], F32, kind="ExternalOutput").ap()

    with tile.TileContext(nc) as tc:
        for _ in range(n_iters):
            _body(nc, tc, d, out_d)
    nc.compile()
    return nc


def _body(nc, tc, d, out_d):
    dma = nc.sync.dma_start

    from contextlib import ExitStack
    octx = ExitStack()
    pp = octx.enter_context(tc.tile_pool(name="persist", bufs=1))

    def T(shape, dtype, name):
        return pp.tile(shape, dtype, tag=name, name=name)

    # ---- persistent SBUF arrays -------------------------------------------
    kT = T([P, NDC * N], BF16, "kT_sb")          # [dh-chunk i][:, i*N + k]
    qT = T([P, NDC * NQ], BF16, "qT_sb")         # [:, i*NQ + q]
    vN = T([P, NKT * HDH], BF16, "v_sb")         # [kt][:, kt*HDH + hd]
    attT = T([P, NKT * 2048], BF16, "attT_sb")   # per kt: h*256 + q

    ident_f = T([P, P], F32, "ident_f")
    ident_b = T([P, P], BF16, "ident_b")
    make_identity(nc, ident_f)
    make_identity(nc, ident_b)

    ones16 = T([P, 1], BF16, "ones16")           # den lhsT: cancels sigma_v=16
    nc.vector.memset(ones16, 16.0)
    epsT = T([P, 1], F32, "epsT")
    nc.vector.memset(epsT, EPS)


    # z2T lives from phase 2 until the end of the logits loop
    z2ctx = ExitStack()
    z2pp = z2ctx.enter_context(tc.tile_pool(name="z2t_pool", bufs=1))
    z2T = z2pp.tile([P, NKT * 2048], BF16, tag="z2T_sb", name="z2T_sb")

    # ---- phases 0-2 in ONE scope: LayerNorm + q/k/v projections + pair
    # MLP share PSUM pools that coexist, so Tile can overlap the PE-bound
    # projections with the ACT-bound pair-MLP gelu stream.
    with tc.tile_pool(name="p01_sbuf", bufs=2) as sb, \
         tc.tile_pool(name="p01_anT", bufs=1) as anp, \
         tc.tile_pool(name="p01_w", bufs=1) as wpool, \
         tc.tile_pool(name="p2_sbuf", bufs=1) as sb2, \
         tc.tile_pool(name="p2_g", bufs=4) as gpool, \
         tc.tile_pool(name="p2_z2e", bufs=2) as z2epool, \
         tc.tile_pool(name="p01_psum", bufs=2, space="PSUM") as ps, \
         tc.tile_pool(name="p2_z1p", bufs=2, space="PSUM") as z1ps, \
         tc.tile_pool(name="p2_z2p", bufs=1, space="PSUM") as z2ps:

        # pair-MLP inputs first on the gpsimd queue (z1 is early PE work)
        rq_sb = sb2.tile([64, NPAIR * P], BF16, name="rq_sb")
        nc.gpsimd.dma_start(out=rq_sb, in_=d["RQ2"])
        bkt_sb = sb2.tile([64, N], BF16, name="bkt_sb")
        nc.gpsimd.dma_start(out=bkt_sb, in_=d["BkT2"])
        w2bd_sb = sb2.tile([P, 32], BF16, name="w2bd_sb")
        dma(out=w2bd_sb, in_=d["W2bd"])
        bp1p_sb = sb2.tile([P, 1], F32, name="bp1p_sb")
        dma(out=bp1p_sb, in_=d["bp1p"])
        bp2p_sb = sb2.tile([P, 1], F32, name="bp2p_sb")
        dma(out=bp2p_sb, in_=d["bp2p"])

        anT = anp.tile([P, 2 * N], BF16, tag="anT", name="anT_sb")
        wq_sb = wpool.tile([P, 2 * HDH], BF16, name="wq_sb")
        wk_sb = wpool.tile([P, 2 * HDH], BF16, name="wk_sb")
        wv_sb = wpool.tile([P, 2 * HDH], BF16, name="wv_sb")
        for ct in range(2):
            nc.gpsimd.dma_start(out=wk_sb[:, ct * HDH:(ct + 1) * HDH],
                                in_=d["Wk"][ct * P:(ct + 1) * P, :])
            nc.gpsimd.dma_start(out=wq_sb[:, ct * HDH:(ct + 1) * HDH],
                                in_=d["Wq"][ct * P:(ct + 1) * P, :])
            nc.gpsimd.dma_start(out=wv_sb[:, ct * HDH:(ct + 1) * HDH],
                                in_=d["Wv"][ct * P:(ct + 1) * P, :])
        bq_sb = wpool.tile([P, NDC], F32, name="bq_sb")
        dma(out=bq_sb, in_=d["biasQ"])

        # LayerNorm (natural layout) + transpose into anT (bf16)
        _mark(nc, 'LN')
        with tc.tile_pool(name="a_pool", bufs=1) as apool:
            af = d["a_full"]
            for hh in range(2):
                a_half = apool.tile([P, NKT // 2 * C], F32, tag="a_all",
                                    name="a_half")
                nc.sync.dma_start(
                    out=a_half,
                    in_=bass.AP(tensor=af.tensor,
                                offset=af.offset + hh * (N // 2) * C,
                                ap=[[C, P], [P * C, NKT // 2], [1, C]]))
                for nt4 in range(NKT // 2):
                    nt = hh * 4 + nt4
                    at = a_half[:, nt4 * C:(nt4 + 1) * C]
                    stats = sb.tile([P, 6], F32, tag="stats")
                    nc.vector.bn_stats(out=stats, in_=at)
                    mv = sb.tile([P, 2], F32, tag="mv")
                    nc.vector.bn_aggr(out=mv, in_=stats)
                    std = sb.tile([P, 1], F32, tag="std")
                    nc.scalar.activation(out=std, in_=mv[:, 1:2], func=AF.Sqrt,
                                         bias=epsT, scale=1.0)
                    rstd = sb.tile([P, 1], F32, tag="rstd")
                    nc.vector.reciprocal(out=rstd, in_=std)
                    an = sb.tile([P, C], BF16, tag="an_t")
                    nc.vector.tensor_scalar(out=an, in0=at, scalar1=mv[:, 0:1],
                                            scalar2=rstd, op0=ALU.subtract,
                                            op1=ALU.mult)
                    # xbar transpose: chunk ct lands at anT[:, ct*N + nt*128]
                    asl = anT[:, nt * P: nt * P + 1]
                    aob = bass.AP(tensor=asl.tensor, offset=asl.offset,
                                  ap=[asl.ap[0], [N, 2], [1, P]])
                    nc.sync.dma_start_transpose(aob, an)

        # ---- pair MLP -> z2T: emitted BEFORE the projections so the PE
        # starts on z1 immediately (projections wait on LayerNorm anyway).
        _mark(nc, 'pair')
        g_tiles = [None] * 4  # rotating per quad
        for tau in range(NTAU):
            for pj in range(2):           # two row-packed pairs per quad
                pi = tau * 2 + pj         # pair index
                for j in range(2):
                    g = gpool.tile([P, N], BF16, tag="g")
                    z1p = z1ps.tile([P, N], F32, tag="z1", name="z1p")
                    for nk in range(2):
                        nc.tensor.matmul(
                            z1p[:, nk * 512:(nk + 1) * 512],
                            lhsT=rq_sb[32 * j:32 * (j + 1), pi * P:(pi + 1) * P],
                            rhs=bkt_sb[32 * j:32 * (j + 1),
                                       nk * 512:(nk + 1) * 512],
                            start=True, stop=True,
                            tile_position=(32 * j, 0))
                    nc.scalar.activation(out=g, in_=z1p,
                                         func=GELU,
                                         bias=bp1p_sb, scale=1.0 / 32.0)
                    g_tiles[pj * 2 + j] = g
            z2p = z2ps.tile([P, N], F32, tag="z2p")
            for j in range(4):
                for nk in range(2):
                    nc.tensor.matmul(
                        z2p[32 * j:32 * (j + 1), nk * 512:(nk + 1) * 512],
                        lhsT=w2bd_sb,
                        rhs=g_tiles[j][:, nk * 512:(nk + 1) * 512],
                        start=True, stop=True,
                        tile_position=(0, 32 * j),
                        skip_group_check=True)
            z2e = z2epool.tile([P, N], BF16, tag="z2e")
            nc.vector.tensor_scalar_add(out=z2e, in0=z2p, scalar1=bp2p_sb)
            # one xbar transpose scatters all 8 k-chunks: out chunk kt is
            # z2T[:, kt*2048 + tau*128 : ... + 128] = z2e[:, kt*128:...].T
            zsl = z2T[:, tau * P: tau * P + 1]
            zob = bass.AP(tensor=zsl.tensor, offset=zsl.offset,
                          ap=[zsl.ap[0], [2048, NKT], [1, P]])
            nc.sync.dma_start_transpose(zob, z2e)

        # kT: per dh-chunk i -> [128, N]   (no bias: q.bk is constant over the
        # softmax axis, so biasK drops out of the attention exactly)
        _mark(nc, 'kT')
        for i in range(NDC):
            for nk in range(2):
                kp = ps.tile([P, 512], F32, tag="proj", name="kp")
                for ct in range(2):
                    nc.tensor.matmul(
                        kp,
                        lhsT=wk_sb[:, ct * HDH + i * P: ct * HDH + (i + 1) * P],
                        rhs=anT[:, ct * N + nk * 512: ct * N + (nk + 1) * 512],
                        start=(ct == 0), stop=(ct == 1))
                nc.vector.tensor_copy(
                    out=kT[:, i * N + nk * 512: i * N + (nk + 1) * 512],
                    in_=kp)

        # qT: per dh-chunk i -> [128, NQ]  (queries are first NQ columns)
        _mark(nc, 'qT')
        for i in range(NDC):
            qp = ps.tile([P, NQ], F32, tag="proj", name="qp")
            for ct in range(2):
                nc.tensor.matmul(
                    qp, lhsT=wq_sb[:, ct * HDH + i * P: ct * HDH + (i + 1) * P],
                    rhs=anT[:, ct * N: ct * N + NQ],
                    start=(ct == 0), stop=(ct == 1))
            nc.vector.tensor_scalar_add(
                out=qT[:, i * NQ:(i + 1) * NQ], in0=qp,
                scalar1=bq_sb[:, i:i + 1])

        # v (natural layout): per key tile kt -> [128, HDH]  (bv is folded into
        # apb host-side: softmax weights sum to 1, so att@(v+bv) = att@v + bv)
        _mark(nc, 'v')
        for kt in range(NKT):
            for dq in range(4):
                vp = ps.tile([P, 512], F32, tag="proj", name="vp")
                for ct in range(2):
                    nc.tensor.matmul(
                        vp,
                        lhsT=anT[:, ct * N + kt * P: ct * N + (kt + 1) * P],
                        rhs=wv_sb[:, ct * HDH + dq * 512:
                                  ct * HDH + (dq + 1) * 512],
                        start=(ct == 0), stop=(ct == 1))
                nc.vector.tensor_copy(
                    out=vN[:, kt * HDH + dq * 512: kt * HDH + (dq + 1) * 512],
                    in_=vp)

    # ---- phase 3: logits, softmax, AV ------------------------------------
    with tc.tile_pool(name="p3_sbuf", bufs=1) as sb3:
        distK_sb = sb3.tile([4, N], BF16, name="distK_sb")
        dma(out=distK_sb, in_=d["distK"])
        distQ8_sb = sb3.tile([4, H * NQ], BF16, name="distQ8_sb")
        dma(out=distQ8_sb, in_=d["distQ8"])

        _mark(nc, 'logits')
        with tc.tile_pool(name="p3_qp", bufs=2, space="PSUM") as qps:
            for kt in range(NKT):
                Qp = qps.tile([P, H * NQ], F32, tag="Qp")
                for cch in range(4):
                    nc.tensor.matmul(
                        Qp[:, cch * 512:(cch + 1) * 512],
                        lhsT=distK_sb[:, kt * P:(kt + 1) * P],
                        rhs=distQ8_sb[:, cch * 512:(cch + 1) * 512],
                        start=True, stop=False)
                for h in range(H):
                    zr = z2T[:, kt * 2048 + 4 * h: kt * 2048 + 4 * h + 1]
                    zap = bass.AP(
                        tensor=zr.tensor, offset=zr.offset,
                        ap=[zr.ap[0], [128, NTAU], [32, 4], [1, 4]])
                    nc.tensor.matmul(
                        Qp[:, h * NQ:(h + 1) * NQ], lhsT=ident_b, rhs=zap,
                        start=False, stop=False)
                for h in range(H):
                    for i2 in range(2):
                        i = h * 2 + i2
                        nc.tensor.matmul(
                            Qp[:, h * NQ:(h + 1) * NQ],
                            lhsT=kT[:, i * N + kt * P: i * N + (kt + 1) * P],
                            rhs=qT[:, i * NQ:(i + 1) * NQ],
                            start=False, stop=(i2 == 1 and h % 2 == 1))
                nc.scalar.activation(
                    out=attT[:, kt * 2048:(kt + 1) * 2048], in_=Qp,
                    func=AF.Exp, bias=0.0, scale=1.0 / 1024.0)

    z2ctx.close()
    avp = octx.enter_context(tc.tile_pool(name="av_persist", bufs=1))
    oT = avp.tile([P, NDC * NQ], BF16, tag="oT_sb", name="oT_sb")
    rden = avp.tile([1, H * NQ], BF16, tag="rden", name="rden_sb")
    rdb = avp.tile([P, H * NQ], BF16, tag="rdb", name="rdb_sb")
    # tail weights, loaded here so the DMA hides under den/AV
    wo_sb = avp.tile([P, NDC * C], BF16, tag="wo_sb", name="wo_sb")
    wod = d["Wo"]
    nc.gpsimd.dma_start(
        out=wo_sb, in_=bass.AP(tensor=wod.tensor, offset=wod.offset,
                               ap=[[C, P], [P * C, NDC], [1, C]]))
    wf1_sb = avp.tile([P, 2 * C], F32, tag="wf1_sb", name="wf1_sb")
    wf2_sb = avp.tile([P, 2 * C], F32, tag="wf2_sb", name="wf2_sb")
    for wname, wsb in (("Wf1", wf1_sb), ("Wf2", wf2_sb)):
        wd = d[wname]
        nc.gpsimd.dma_start(
            out=wsb, in_=bass.AP(tensor=wd.tensor, offset=wd.offset,
                                 ap=[[C, P], [P * C, 2], [1, C]]))

    _mark(nc, 'den_av')
    with tc.tile_pool(name="p3_den", bufs=1, space="PSUM") as denps, \
         tc.tile_pool(name="p3_av", bufs=4, space="PSUM") as avps:
        denp = denps.tile([1, H * NQ], F32, name="denp")
        for cc in range(4):
            for kt in range(NKT):
                nc.tensor.matmul(
                    denp[:, cc * 512:(cc + 1) * 512], lhsT=ones16,
                    rhs=attT[:, kt * 2048 + cc * 512: kt * 2048 + (cc + 1) * 512],
                    start=(kt == 0), stop=(kt == NKT - 1))
        with nc.allow_low_precision(reason="softmax denom bcast in bf16"):
            nc.vector.reciprocal(out=rden, in_=denp)
        rdd = d["rden_dram"]
        dma(out=rdd, in_=rden)
        nc.gpsimd.dma_start(
            out=rdb, in_=bass.AP(tensor=rdd.tensor, offset=rdd.offset,
                                 ap=[[0, P], [1, H * NQ]]))

        for h in range(H):
            for dhh in range(2):
                i = h * 2 + dhh
                op = avps.tile([P, NQ], F32, tag="op")
                for kt in range(NKT):
                    nc.tensor.matmul(
                        op,
                        lhsT=vN[:, kt * HDH + h * DH + dhh * P:
                                kt * HDH + h * DH + (dhh + 1) * P],
                        rhs=attT[:, kt * 2048 + h * NQ:
                                 kt * 2048 + (h + 1) * NQ],
                        start=(kt == 0), stop=(kt == NKT - 1))
                nc.vector.tensor_tensor(
                    out=oT[:, i * NQ:(i + 1) * NQ], in0=op,
                    in1=rdb[:, h * NQ:(h + 1) * NQ], op=ALU.mult)

    _mark(nc, 'tail')
    # ---- phase 4: output projection + residual + FFN ---------------------
    with tc.tile_pool(name="p4_sbuf", bufs=1) as sb4, \
         tc.tile_pool(name="p4_ps", bufs=2, space="PSUM") as ps4, \
         tc.tile_pool(name="p4_pst", bufs=4, space="PSUM") as pst4:

        apb_sb = sb4.tile([P, 2 * C], F32, name="apb_sb")
        apd = d["apb"]
        nc.gpsimd.dma_start(
            out=apb_sb, in_=bass.AP(tensor=apd.tensor, offset=apd.offset,
                                    ap=[[C, P], [P * C, 2], [1, C]]))
        bf1p_sb = sb4.tile([P, C // P], F32, name="bf1p_sb")
        dma(out=bf1p_sb, in_=d["bf1p"])
        bf2b = sb4.tile([P, C], F32, name="bf2b")
        srcb = d["bf2r"]
        nc.gpsimd.dma_start(
            out=bf2b, in_=bass.AP(tensor=srcb.tensor, offset=srcb.offset,
                                  ap=[[0, P], [1, C]]))

        res = sb4.tile([P, 2 * C], F32, name="res_sb")      # [qh][:, qh*C + c]
        for qh in range(2):
            prj = ps4.tile([P, C], F32, tag="p4")
            for i in range(NDC):
                nc.tensor.matmul(
                    prj, lhsT=oT[:, i * NQ + qh * P: i * NQ + (qh + 1) * P],
                    rhs=wo_sb[:, i * C:(i + 1) * C],
                    start=(i == 0), stop=(i == NDC - 1))
            nc.vector.tensor_tensor(
                out=res[:, qh * C:(qh + 1) * C], in0=prj,
                in1=apb_sb[:, qh * C:(qh + 1) * C], op=ALU.add)

        resT = sb4.tile([P, 2 * NQ], F32, name="resT_sb")   # [ct][:, ct*NQ + q]
        for qh in range(2):
            for ct in range(2):
                tp4 = pst4.tile([P, P], F32, tag="tp4")
                nc.tensor.transpose(
                    tp4, res[:, qh * C + ct * P: qh * C + (ct + 1) * P],
                    ident_f)
                nc.vector.tensor_copy(
                    out=resT[:, ct * NQ + qh * P: ct * NQ + (qh + 1) * P],
                    in_=tp4)

        gT = sb4.tile([P, 2 * NQ], F32, name="gT_sb")       # [cc][:, cc*NQ + q]
        for cc in range(2):
            fp = ps4.tile([P, NQ], F32, tag="p4")
            for ct in range(2):
                nc.tensor.matmul(
                    fp, lhsT=wf1_sb[:, ct * C + cc * P: ct * C + (cc + 1) * P],
                    rhs=resT[:, ct * NQ:(ct + 1) * NQ],
                    start=(ct == 0), stop=(ct == 1))
            nc.scalar.activation(out=gT[:, cc * NQ:(cc + 1) * NQ], in_=fp,
                                 func=GELU,
                                 bias=bf1p_sb[:, cc:cc + 1], scale=1.0)

        for qh in range(2):
            f2 = ps4.tile([P, C], F32, tag="p4")
            for cc in range(2):
                nc.tensor.matmul(
                    f2, lhsT=gT[:, cc * NQ + qh * P: cc * NQ + (qh + 1) * P],
                    rhs=wf2_sb[:, cc * C:(cc + 1) * C],
                    start=(cc == 0), stop=(cc == 1))
            ot = sb4.tile([P, C], F32, tag="ot")
            nc.vector.tensor_tensor(out=ot, in0=f2, in1=bf2b, op=ALU.add)
            dma(out=out_d[qh * P:(qh + 1) * P, :], in_=ot)

    octx.close()


# ---------------------------------------------------------------------------
# host-side input prep
# ---------------------------------------------------------------------------

def _prep_core_inputs(inputs):
    f32 = np.float32
    p = np.asarray(inputs["p"], f32)
    a = np.asarray(inputs["a"], f32)
    sigma = float(np.asarray(inputs["window_size"]).reshape(-1)[0])
    ln_s = np.asarray(inputs["ln_scale"], f32)
    ln_b = np.asarray(inputs["ln_bias"], f32)
    Wq, bq = np.asarray(inputs["Wq"], f32), np.asarray(inputs["bq"], f32)
    Wk, bk = np.asarray(inputs["Wk"], f32), np.asarray(inputs["bk"], f32)
    Wv, bv = np.asarray(inputs["Wv"], f32), np.asarray(inputs["bv"], f32)
    rff_B = np.asarray(inputs["rff_B"], f32)
    Wp1, bp1 = np.asarray(inputs["Wp1"], f32), np.asarray(inputs["bp1"], f32)
    Wp2, bp2 = np.asarray(inputs["Wp2"], f32), np.asarray(inputs["bp2"], f32)
    Wo, bo = np.asarray(inputs["Wo"], f32), np.asarray(inputs["bo"], f32)
    Wf1, bf1 = np.asarray(inputs["Wf1"], f32), np.asarray(inputs["bf1"], f32)
    Wf2, bf2 = np.asarray(inputs["Wf2"], f32), np.asarray(inputs["bf2"], f32)

    # operand scaling: q,k carry sigma_q=sigma_k=8, v carries sigma_v=16.
    # The logits PSUM is then LAM=1024x the true logits (1/sqrt(DH)=1/16
    # folded in), compensated by the Exp activation's scale=1/LAM.  The
    # denominator matmul uses lhsT=16 so op*rdb cancels sigma_v exactly.
    SQ, SK, SV = 8.0, 8.0, 16.0
    LAM = SQ * SK * np.sqrt(f32(DH))       # 1024
    Wq_f = (ln_s[:, None] * Wq) * SQ
    bq_f = (bq + ln_b @ Wq) * SQ
    Wk_f = (ln_s[:, None] * Wk) * SK
    Wv_f = (ln_s[:, None] * Wv) * SV
    bv_f = bv + ln_b @ Wv

    u = 2.0 * np.pi * (p @ rff_B)          # [B, N, F]
    su, cu = np.sin(u), np.cos(u)
    pn2 = (p ** 2).sum(-1)                 # [B, N]

    # W2 block-diag: rows (ql*32+h1), cols (h*4+ql), scaled by LAM
    W2bd = np.zeros((P, 32), f32)
    for ql in range(4):
        for h1 in range(PH):
            for h in range(H):
                W2bd[ql * 32 + h1, h * 4 + ql] = Wp2[h1, h] * LAM
    bp1p = np.tile(bp1, 4).reshape(P, 1).astype(f32)
    bp2p = (np.tile(np.repeat(bp2, 4), 4) * LAM).reshape(P, 1).astype(f32)
    bf1p = bf1.reshape(C // P, P).T.copy()
    biasQ = bq_f.reshape(NDC, P).T.copy()

    shared = {
        "Wq": Wq_f.astype(_BF), "Wk": Wk_f.astype(_BF),
        "Wv": Wv_f.astype(_BF),
        "biasQ": biasQ,
        "W2bd": W2bd.astype(_BF), "bp1p": bp1p, "bp2p": bp2p,
        "Wo": Wo.astype(_BF), "Wf1": Wf1, "Wf2": Wf2,
        "bf1p": bf1p, "bf2r": bf2.reshape(1, C),
    }

    in_maps = []
    for c in range(NCORES):
        b, t = c // 4, c % 4
        perm = (np.arange(N) + t * NQ) % N
        a_p = a[b][perm]
        su_p, cu_p = su[b][perm], cu[b][perm]
        p_p, pn2_p = p[b][perm], pn2[b][perm]

        # k-side of z1 trig expansion: rows j'=f -> cos, j'=F+f -> sin
        BkT = np.concatenate([cu_p.T, su_p.T], 0)       # [32, N]
        BkT2 = np.concatenate([BkT, BkT], 0)            # [64, N] row-packed x2

        # q-side: R[q, j', h1], pre-scaled by MU=32 (the gelu reads the z1
        # PSUM with scale=1/MU; harmless in bf16, needed if z1 goes fp8)
        MU = 32.0
        suq, cuq = su_p[:NQ], cu_p[:NQ]                 # [NQ, F]
        Ra = (np.einsum("qf,fh->qfh", suq, Wp1[:F]) +
              np.einsum("qf,fh->qfh", cuq, Wp1[F:]))
        Rb = (np.einsum("qf,fh->qfh", -cuq, Wp1[:F]) +
              np.einsum("qf,fh->qfh", suq, Wp1[F:]))
        R = np.concatenate([Ra, Rb], 1) * MU            # [NQ, 32, PH]
        # RQ2[32j+j', pi*128 + ql*32 + h1], q = pi*8 + j*4 + ql
        RQ2 = (R.reshape(NPAIR, 2, 4, 32, PH)
               .transpose(1, 3, 0, 2, 4).reshape(64, NPAIR * P))

        # rank-4 gaussian-window term in bf16 (fp32 matmuls are 4x slower);
        # row 3 keeps exp() <= e^~1 per row
        inv2s = 1.0 / (2.0 * sigma * sigma)
        distK = np.stack([p_p[:, 0], p_p[:, 1], pn2_p,
                          np.ones(N, f32)], 0)          # [4, N]
        distQ = np.stack([p_p[:NQ, 0] * (2.0 * inv2s * LAM),
                          p_p[:NQ, 1] * (2.0 * inv2s * LAM),
                          -np.full(NQ, inv2s * LAM, f32),
                          -pn2_p[:NQ] * inv2s * LAM], 0)  # [4, NQ]
        distQ8 = np.tile(distQ, (1, H))                 # col h*NQ+q

        m = dict(shared)
        m.update({
            "a_full": np.ascontiguousarray(a_p),
            "RQ2": RQ2.astype(_BF), "BkT2": np.ascontiguousarray(BkT2).astype(_BF),
            "distK": np.ascontiguousarray(distK).astype(_BF),
            "distQ8": np.ascontiguousarray(distQ8).astype(_BF),
            "apb": np.ascontiguousarray(a_p[:NQ] + bo + bv_f @ Wo),
        })
        in_maps.append({k: np.ascontiguousarray(v) for k, v in m.items()})
    return in_maps


# ---------------------------------------------------------------------------
# entry point
# ---------------------------------------------------------------------------

_NC_CACHE = None


def _get_nc():
    global _NC_CACHE
    if _NC_CACHE is None:
        _NC_CACHE = _build_program()
    return _NC_CACHE


def kernel(**inputs):
    from concourse import bass_utils
    in_maps = _prep_core_inputs(inputs)
    nc = _get_nc()
    res = bass_utils.run_bass_kernel_spmd(nc, in_maps,
                                          core_ids=list(range(NCORES)))
    out = np.empty((B, N, C), np.float32)
    for c in range(NCORES):
        b, t = c // 4, c % 4
        out[b, t * NQ:(t + 1) * NQ, :] = res.results[c]["out"]
    return out
